# revision 2
# baseline (speedup 1.0000x reference)
"""Trainium2 Bass kernel for nn_ChoquetIntegralConstrained.

Computes: sigmoid((x @ w_eff) / weight_sum - thr) where w_eff is built from
(wc, wint) via the constraint transform, x is [16384, 8256] f32.

Strategy: pure data parallel over batch across 8 NeuronCores. Each core gets
2048 rows, processed as 16 tiles of [128 rows, 8256]. The dot product with the
replicated weight vector is one fused DVE tensor_tensor_reduce per tile
(out = x*w, accum_out = row-sum), which hides entirely under the HBM-bound
x DMA stream. The tiny constraint transform on the 8256 weights is done on the
host in fp32 (identical elementwise semantics to the reference).
"""

import sys

import numpy as np

sys.path.insert(0, "/opt/trn_rl_repo")

N_CRIT = 128
N_PAIRS = N_CRIT * (N_CRIT - 1) // 2  # 8128
D = N_CRIT + N_PAIRS  # 8256
BATCH = 16384
N_CORES = 8
ROWS_PER_CORE = BATCH // N_CORES  # 2048
P = 128  # SBUF partitions
TILES_PER_CORE = ROWS_PER_CORE // P  # 16
MIN_W = np.float32(1e-07)

_CACHE = {}


def _build_program():
    import concourse.tile as tile
    from concourse import bacc, mybir

    nc = bacc.Bacc(
        "TRN2",
        debug=False,
        target_bir_lowering=False,
        num_devices=N_CORES,
    )
    f32 = mybir.dt.float32
    x_d = nc.dram_tensor("x", [ROWS_PER_CORE, D], f32, kind="ExternalInput").ap()
    w_d = nc.dram_tensor("wrep", [P, D], f32, kind="ExternalInput").ap()
    c_d = nc.dram_tensor("consts", [P, 2], f32, kind="ExternalInput").ap()
    y_d = nc.dram_tensor("y", [P, TILES_PER_CORE], f32, kind="ExternalOutput").ap()

    with tile.TileContext(nc) as tc:
        with (
            tc.tile_pool(name="xp", bufs=3) as xp,
            tc.tile_pool(name="sp", bufs=1) as sp,
            tc.tile_pool(name="wp", bufs=1) as wp,
        ):
            w_t = wp.tile([P, D], f32)
            nc.sync.dma_start(out=w_t[:], in_=w_d[:])
            c_t = wp.tile([P, 2], f32)
            nc.sync.dma_start(out=c_t[:], in_=c_d[:])
            acc_t = wp.tile([P, TILES_PER_CORE], f32)
            scratch = sp.tile([P, D], f32)

            for t in range(TILES_PER_CORE):
                x_t = xp.tile([P, D], f32)
                nc.sync.dma_start(out=x_t[:], in_=x_d[t * P : (t + 1) * P, :])
                # out = (x * 1.0) * w ; accum_out = row-sum(out)
                nc.vector.scalar_tensor_tensor(
                    out=scratch[:],
                    in0=x_t[:],
                    scalar=1.0,
                    in1=w_t[:],
                    op0=mybir.AluOpType.mult,
                    op1=mybir.AluOpType.mult,
                    accum_out=acc_t[:, t : t + 1],
                )

            y_t = wp.tile([P, TILES_PER_CORE], f32)
            nc.scalar.activation(
                out=y_t[:],
                in_=acc_t[:],
                func=mybir.ActivationFunctionType.Sigmoid,
                bias=c_t[:, 1:2],
                scale=c_t[:, 0:1],
            )
            nc.sync.dma_start(out=y_d[:], in_=y_t[:])

    nc.compile()
    return nc


def _get_program():
    if "nc" not in _CACHE:
        _CACHE["nc"] = _build_program()
    return _CACHE["nc"]


def _host_weight_prep(wc, wint, thr):
    """Mirror reference._constrained_weights + weight_sum in fp32 numpy."""
    wc = np.asarray(wc, dtype=np.float32)
    wint = np.asarray(wint, dtype=np.float32)
    wc_eff = np.where(wc < 0, MIN_W, wc)
    ii, jj = np.triu_indices(N_CRIT, k=1)
    lower = np.maximum(-wc_eff[:, ii], -wc_eff[:, jj])
    wint_eff = np.maximum(wint, lower)
    w_eff = np.concatenate([wc_eff, wint_eff], axis=1)  # [1, D]
    wsum = np.float32(wc_eff.sum(dtype=np.float32)) + np.float32(
        wint_eff.sum(dtype=np.float32)
    )
    inv_wsum = np.float32(1.0) / wsum
    neg_thr = -np.float32(np.asarray(thr).reshape(-1)[0])
    return w_eff, inv_wsum, neg_thr


def _make_in_maps(x, wc, wint, thr):
    x = np.ascontiguousarray(np.asarray(x, dtype=np.float32))
    w_eff, inv_wsum, neg_thr = _host_weight_prep(wc, wint, thr)
    wrep = np.ascontiguousarray(np.broadcast_to(w_eff, (P, D)))
    consts = np.empty((P, 2), dtype=np.float32)
    consts[:, 0] = inv_wsum
    consts[:, 1] = neg_thr
    return [
        {
            "x": np.ascontiguousarray(x[c * ROWS_PER_CORE : (c + 1) * ROWS_PER_CORE]),
            "wrep": wrep,
            "consts": consts,
        }
        for c in range(N_CORES)
    ]


def _gather(results):
    # y core tile is [P, TILES]: y[p, t] = batch row t*128 + p within the shard
    parts = [
        np.asarray(results[c]["y"]).T.reshape(ROWS_PER_CORE) for c in range(N_CORES)
    ]
    return np.concatenate(parts).reshape(BATCH, 1).astype(np.float32)


def _run(x, wc, wint, thr, trace=False):
    from concourse import bass_utils

    nc = _get_program()
    in_maps = _make_in_maps(x, wc, wint, thr)
    res = bass_utils.run_bass_kernel_spmd(
        nc, in_maps, core_ids=list(range(N_CORES)), trace=trace
    )
    return _gather(res.results), res


def kernel(x, wc, wint, thr):
    out, _ = _run(x, wc, wint, thr, trace=False)
    return out
